# revision 54
# baseline (speedup 1.0000x reference)
"""Sliding-window (banded causal) multi-head attention on 8 TRN2 NeuronCores.

Sharding: 8 cores = 2 batches x 4 head-groups (4 heads of 64 dims each).
Each core computes QKV projections for its 4 heads, RoPE, banded flash
attention (window 1024), and a partial output projection (its 256 columns
of wo). The host sums the 4 partial outputs per batch element.

Device layout choices (per core):
  - everything bf16 on the TensorEngine, fp32 accumulation in PSUM
  - activations pre-transposed on host: xT [1024, 2048] so the in-dim is
    the contraction (partition) axis
  - Q/K produced directly transposed: QT/KT [256 outdim, 2048 tok]
  - scores computed transposed, [k, q] blocks, so probs feed the PV matmul
    as the moving operand with V as the stationary operand (no on-chip
    transposes anywhere)
  - softmax denominators come free from a ones-column appended to V
  - no running-max subtraction: logits are O(1) here, exp is safe
  - schedule: dense projection phase (Q/K interleaved over 8 PSUM banks,
    streaming against the x load), then two software-pipelined attention
    passes (scores of q-tile i+1 are emitted before the PV matmuls of
    q-tile i so the exp stream on ScalarE never starves); normalization
    is batched off the critical chain and the output projection + stores
    trail inside pass B
"""

from contextlib import ExitStack

import numpy as np
import ml_dtypes

import concourse.bass as bass
import concourse.tile as tile
from concourse import bacc, mybir
from concourse.bass_utils import run_bass_kernel_spmd

BF16 = mybir.dt.bfloat16
F32 = mybir.dt.float32
F32R = mybir.dt.float32r

B, S, H = 2, 2048, 1024
NH, HD = 16, 64
WINDOW = 1024
ROPE_THETA = 10000.0
MAX_POS = 2048
N_CORES = 8
HG = 4                      # heads per core
GD = HG * HD                # 256: head-group dim per core
P = 128
NQT = S // P                # 16 q tiles
WT = WINDOW // P            # 8: window in tiles
CH = H // P                 # 8 contraction chunks
VW = HD + 1                 # 65: V width with ones column

_cache = {}


def _build():
    nc = bacc.Bacc("TRN2", target_bir_lowering=False, debug=False,
                   enable_asserts=False, num_devices=N_CORES)

    xT_d = nc.dram_tensor("xT", [H, S], BF16, kind="ExternalInput")
    wqT_d = nc.dram_tensor("wqT", [H, GD], BF16, kind="ExternalInput")
    wkT_d = nc.dram_tensor("wkT", [H, GD], BF16, kind="ExternalInput")
    wvT_d = nc.dram_tensor("wvT", [H, GD], BF16, kind="ExternalInput")
    woT_d = nc.dram_tensor("woT", [GD, H], BF16, kind="ExternalInput")
    cosT_d = nc.dram_tensor("cosT", [P, S], BF16, kind="ExternalInput")
    sinTs_d = nc.dram_tensor("sinTs", [P, S], BF16, kind="ExternalInput")
    bq_d = nc.dram_tensor("bq2", [P, 2], F32, kind="ExternalInput")
    bk_d = nc.dram_tensor("bk2", [P, 2], F32, kind="ExternalInput")
    bqs_d = nc.dram_tensor("bq2s", [P, 2], F32, kind="ExternalInput")
    bks_d = nc.dram_tensor("bk2s", [P, 2], F32, kind="ExternalInput")
    # combined [diag | far] edge masks, bf16 0/1
    masks_d = nc.dram_tensor("masks", [P, 2 * P], BF16, kind="ExternalInput")
    out_d = nc.dram_tensor("out", [S, H], F32, kind="ExternalOutput")

    with tile.TileContext(nc) as tc, ExitStack() as ctx:
        const = ctx.enter_context(tc.tile_pool(name="const", bufs=1))
        qk = ctx.enter_context(tc.tile_pool(name="qk", bufs=1))
        vp = ctx.enter_context(tc.tile_pool(name="vp", bufs=1))
        pp = ctx.enter_context(tc.tile_pool(name="pp", bufs=6))
        cxp = ctx.enter_context(tc.tile_pool(name="cxp", bufs=1))
        osb = ctx.enter_context(tc.tile_pool(name="osb", bufs=2))
        sm = ctx.enter_context(tc.tile_pool(name="sm", bufs=3))

        # ---- merged loads, ordered by first use on one queue ----
        wq_sb = const.tile([P, CH * GD], BF16, name="wq_sb")
        wk_sb = const.tile([P, CH * GD], BF16, name="wk_sb")
        wv_sb = const.tile([P, CH * GD], BF16, name="wv_sb")
        wo_sb = const.tile([P, 2 * H], BF16, name="wo_sb")
        x_sb = const.tile([P, CH * S], BF16, name="x_sb")
        cosT = const.tile([P, S], BF16, name="cosT")
        sinTs = const.tile([P, S], BF16, name="sinTs")
        bq_sb = const.tile([P, 2], F32, name="bq_sb")
        bk_sb = const.tile([P, 2], F32, name="bk_sb")
        bqs_sb = const.tile([P, 2], F32, name="bqs_sb")
        bks_sb = const.tile([P, 2], F32, name="bks_sb")
        masks = const.tile([P, 2 * P], BF16, name="masks")

        def chunked(dram, w):
            return dram.ap().rearrange("(c p) w -> p c w", p=P)

        xv = x_sb.rearrange("p (c w) -> p c w", c=CH)
        xs = chunked(xT_d, S)
        nc.scalar.dma_start(wq_sb.rearrange("p (c w) -> p c w", c=CH),
                            chunked(wqT_d, GD))
        for lo, hi in ((0, 1), (1, 2), (2, 4), (4, 6), (6, 8)):
            nc.sync.dma_start(xv[:, lo:hi], xs[:, lo:hi])
        nc.scalar.dma_start(wk_sb.rearrange("p (c w) -> p c w", c=CH),
                            chunked(wkT_d, GD))
        nc.scalar.dma_start(wv_sb.rearrange("p (c w) -> p c w", c=CH),
                            chunked(wvT_d, GD))
        nc.scalar.dma_start(cosT[:], cosT_d.ap())
        nc.scalar.dma_start(sinTs[:], sinTs_d.ap())
        nc.scalar.dma_start(wo_sb.rearrange("p (c w) -> p c w", c=2),
                            chunked(woT_d, H))
        nc.scalar.dma_start(bq_sb[:], bq_d.ap())
        nc.scalar.dma_start(bk_sb[:], bk_d.ap())
        nc.scalar.dma_start(bqs_sb[:], bqs_d.ap())
        nc.scalar.dma_start(bks_sb[:], bks_d.ap())
        nc.scalar.dma_start(masks[:], masks_d.ap())

        def xc(c):
            return x_sb[:, c * S:(c + 1) * S]

        def wc(w_sb, c, width=GD):
            return w_sb[:, c * width:(c + 1) * width]

        q_sb = [qk.tile([P, S], BF16, name=f"q{m}") for m in range(2)]
        k_sb = [qk.tile([P, S], BF16, name=f"k{m}") for m in range(2)]
        qs_sb = [qk.tile([P, S], BF16, name=f"qs{m}") for m in range(2)]
        ks_sb = [qk.tile([P, S], BF16, name=f"ks{m}") for m in range(2)]
        v_sb = [vp.tile([P, HG * VW], BF16, name=f"v{t}") for t in range(NQT)]
        ctx_sb = [cxp.tile([P, S], BF16, name=f"cx{m}") for m in range(2)]
        cx_raw = [cxp.tile([VW, 2 * S], BF16, name=f"cxr{m}") for m in range(2)]
        _osb = [osb.tile([P, H], F32, tag="osb", name=f"ot{t}", bufs=4)
                for t in range(NQT)]

        def proj_qk_c_outer(pool, tag, w_sb, dest, b_sb, m, w2=None,
                            dest2=None):
            # c-outer: starts on the first x chunk. With w2/dest2 the two
            # projections interleave (16 MMs per x chunk, 8 psum banks) so
            # the PE keeps pace with the streaming x load.
            pairs = [(w_sb, dest)] + ([(w2, dest2)] if w2 is not None else [])
            pss = [[pool.tile([P, 512], F32, tag=tag, name=f"pj{m}{n}{i}")
                    for n in range(4)] for i in range(len(pairs))]
            for c in range(CH):
                for i, (w, _) in enumerate(pairs):
                    for n in range(4):
                        nc.tensor.matmul(
                            pss[i][n][:], wc(w, c)[:, m * P:(m + 1) * P],
                            xc(c)[:, n * 512:(n + 1) * 512],
                            start=(c == 0), stop=(c == CH - 1))
            for i, (_, d) in enumerate(pairs):
                for n in range(4):
                    nc.scalar.copy(d[m][:, n * 512:(n + 1) * 512],
                                   pss[i][n][:])

        def proj_qk_c_inner(pool, tag, w_sb, dest, b_sb, m):
            # c-inner: one psum bank at a time (for the overlapped m=1 pass)
            for n in range(4):
                ps = pool.tile([P, 512], F32, tag=tag, name=f"pj1{m}{n}")
                for c in range(CH):
                    nc.tensor.matmul(
                        ps[:], wc(w_sb, c)[:, m * P:(m + 1) * P],
                        xc(c)[:, n * 512:(n + 1) * 512],
                        start=(c == 0), stop=(c == CH - 1))
                nc.vector.tensor_scalar_add(
                    dest[m][:, n * 512:(n + 1) * 512], ps[:],
                    b_sb[:, m:m + 1])

        def rope(m):
            # column-halved so the first half unblocks attention early;
            # the Q/K biases ride the trig multiplies: (x + b) * cos etc.
            for half in range(2):
                cl = slice(half * (S // 2), (half + 1) * (S // 2))
                for src, shf, bc, bs in ((q_sb, qs_sb, bq_sb, bqs_sb),
                                         (k_sb, ks_sb, bk_sb, bks_sb)):
                    for hb in range(2):
                        o = hb * HD
                        nc.sync.dma_start(shf[m][o:o + 32, cl],
                                          src[m][o + 32:o + 64, cl])
                        nc.sync.dma_start(shf[m][o + 32:o + 64, cl],
                                          src[m][o:o + 32, cl])
                    nc.vector.scalar_tensor_tensor(
                        shf[m][:, cl], shf[m][:, cl], bs[:, m:m + 1],
                        sinTs[:, cl],
                        mybir.AluOpType.add, mybir.AluOpType.mult)
                    nc.vector.scalar_tensor_tensor(
                        src[m][:, cl], src[m][:, cl], bc[:, m:m + 1],
                        cosT[:, cl],
                        mybir.AluOpType.add, mybir.AluOpType.mult)
                    nc.vector.tensor_add(src[m][:, cl], src[m][:, cl],
                                         shf[m][:, cl])

        def attn_scores(sp, mt, qi):
            kt0 = max(0, qi - WT)
            nkt = qi - kt0 + 1
            # block order: [diag, far?, middles...]
            kts = [qi]
            n_edge = 1
            if qi >= WT:
                kts.append(kt0)
                n_edge = 2
            kts.extend(range(kt0 + (1 if qi >= WT else 0), qi))

            s_ps = [sp.tile([P, WT * P + P], F32, tag="sp",
                            name=f"sps{mt}{qi}{hb}") for hb in range(2)]
            # interleave the two heads' score matmuls: lhsT base partitions
            # 0/64 give disjoint PE row groups -> concurrent on hardware
            for i, kt in enumerate(kts):
                for hb in range(2):
                    ho = hb * HD
                    nc.tensor.matmul(
                        s_ps[hb][:, i * P:(i + 1) * P],
                        k_sb[mt][ho:ho + HD, kt * P:(kt + 1) * P],
                        q_sb[mt][ho:ho + HD, qi * P:(qi + 1) * P],
                        start=True, stop=True)
            prb = []
            for hb in range(2):
                h = mt * 2 + hb
                probs = pp.tile([P, WT * P + P], BF16, tag="pp",
                                name=f"pr{h}{qi}")
                nc.scalar.activation(
                    probs[:, 0:nkt * P], s_ps[hb][:, 0:nkt * P],
                    mybir.ActivationFunctionType.Exp,
                    scale=float(1.0 / np.sqrt(HD)))
                nc.vector.tensor_mul(
                    probs[:, 0:n_edge * P], probs[:, 0:n_edge * P],
                    masks[:, 0:n_edge * P])
                prb.append(probs)
            return kts, n_edge, prb

        def attn_ctx(cp, mt, qi, kts, n_edge, prb):
            nkt = len(kts)
            ctx_ps = cp.tile([VW, 2 * P], F32, tag="ctx", name=f"cps{mt}{qi}")
            for hb in range(2):
                h = mt * 2 + hb
                # ctx^T [65, q]: middles first, masked edge blocks last
                issue = list(range(n_edge, nkt)) + list(range(n_edge))
                for j, i in enumerate(issue):
                    nc.tensor.matmul(
                        ctx_ps[:, hb * P:(hb + 1) * P],
                        v_sb[kts[i]][:, h * VW:(h + 1) * VW],
                        prb[hb][:, i * P:(i + 1) * P],
                        start=(j == 0), stop=(j == nkt - 1))
            # drain PSUM with one unnormalized copy (incl. the sums row)
            nc.vector.tensor_copy(
                cx_raw[mt][:, qi * 2 * P:(qi + 1) * 2 * P], ctx_ps[:])

        def norm2(mt, qi0):
            # normalize q-tiles qi0, qi0+1 (broadcast + recip + 4 muls)
            c0 = qi0 * 2 * P
            rinv = sm.tile([1, 4 * P], F32, tag="rinv", name=f"ri{mt}{qi0}")
            nc.vector.reciprocal(rinv[:], cx_raw[mt][HD:HD + 1, c0:c0 + 4 * P])
            rbc = sm.tile([P, 4 * P], F32, tag="rbc", name=f"rb{mt}{qi0}")
            nc.gpsimd.partition_broadcast(rbc[:], rinv[:])
            for qj in (qi0, qi0 + 1):
                for hb in range(2):
                    ho = hb * HD
                    nc.gpsimd.tensor_mul(
                        ctx_sb[mt][ho:ho + HD, qj * P:(qj + 1) * P],
                        cx_raw[mt][0:HD, qj * 2 * P + hb * P:
                                   qj * 2 * P + (hb + 1) * P],
                        rbc[0:HD, (qj - qi0) * 2 * P + hb * P:
                            (qj - qi0) * 2 * P + (hb + 1) * P])

        def outproj_t(cp, t):
            o_sb = _osb[t]
            for n in range(2):
                ps = cp.tile([P, 512], F32, tag="ctx", name=f"po{t}{n}")
                for c in range(2):
                    nc.tensor.matmul(
                        ps[:], ctx_sb[c][:, t * P:(t + 1) * P],
                        wc(wo_sb, c, H)[:, n * 512:(n + 1) * 512],
                        start=(c == 0), stop=(c == 1))
                dst_o = o_sb[:, n * 512:(n + 1) * 512]
                if (t + n) % 2 == 0:
                    nc.scalar.copy(dst_o, ps[:])
                else:
                    nc.vector.tensor_copy(dst_o, ps[:])
            e = nc.sync if t % 2 == 0 else nc.scalar
            e.dma_start(out_d.ap()[t * P:(t + 1) * P, :], o_sb[:])

        # ---- phase 1: all projections + V ----
        with tc.tile_pool(name="pj", bufs=8, space="PSUM") as pj:
            proj_qk_c_outer(pj, "pj", wq_sb, q_sb, bq_sb, 0,
                            w2=wk_sb, dest2=k_sb)
            proj_qk_c_outer(pj, "pj", wq_sb, q_sb, bq_sb, 1,
                            w2=wk_sb, dest2=k_sb)
            for t in range(NQT):
                nc.gpsimd.memset(v_sb[t][:], 1.0)
            for t in range(NQT):
                ps = pj.tile([P, GD], F32, tag="pj", name=f"pjv{t}")
                for c in range(CH):
                    nc.tensor.matmul(
                        ps[:], xc(c)[:, t * P:(t + 1) * P], wc(wv_sb, c),
                        start=(c == 0), stop=(c == CH - 1))
                vdst = v_sb[t].rearrange("p (h d) -> p h d", h=HG)[:, :, 0:HD]
                vsrc = ps.rearrange("p (h d) -> p h d", h=HG)
                nc.scalar.copy(vdst, vsrc)
            rope(0)
            rope(1)

        # ---- attention passes (scores of qi+1 emitted before ctx of qi
        # so the PE stream never blocks the next exp) ----
        with tc.tile_pool(name="sp", bufs=2, space="PSUM") as sp, \
             tc.tile_pool(name="cp", bufs=2, space="PSUM") as cp:
            pend = None
            for qi in range(NQT):
                cur = attn_scores(sp, 0, qi)
                if pend is not None:
                    attn_ctx(cp, 0, qi - 1, *pend)
                    if qi % 2 == 0 and qi >= 2:
                        norm2(0, qi - 2)
                pend = cur
            attn_ctx(cp, 0, NQT - 1, *pend)
            norm2(0, NQT - 2)
            # ---- pass B (heads 2,3) + spread output projection ----
            pend = None
            for qi in range(NQT):
                cur = attn_scores(sp, 1, qi)
                if pend is not None:
                    attn_ctx(cp, 1, qi - 1, *pend)
                    if qi % 2 == 0 and qi >= 2:
                        norm2(1, qi - 2)
                    if qi >= 4:
                        outproj_t(cp, qi - 4)
                pend = cur
            attn_ctx(cp, 1, NQT - 1, *pend)
            norm2(1, NQT - 2)
            for t in range(NQT - 4, NQT):
                outproj_t(cp, t)

    nc.compile()
    return nc


def _rope_tables():
    inv_freq = 1.0 / (ROPE_THETA ** (np.arange(0, HD, 2, dtype=np.float64) / HD))
    t = np.arange(MAX_POS, dtype=np.float64)
    freqs = np.outer(t, inv_freq)                       # [MAX_POS, 32]
    emb = np.concatenate([freqs, freqs], axis=-1)       # [MAX_POS, 64]
    return np.cos(emb).astype(np.float32), np.sin(emb).astype(np.float32)


def kernel(hidden_states, position_ids, wq, bq, wk, bk, wv, bv, wo, bo):
    bf16 = ml_dtypes.bfloat16
    if "nc" not in _cache:
        _cache["nc"] = _build()
    nc = _cache["nc"]

    cos_t, sin_t = _rope_tables()
    pos = np.clip(np.asarray(position_ids), 0, MAX_POS - 1).astype(np.int64)

    maskd = np.triu(np.ones((P, P), np.float32))        # k <= q (diag block)
    maskf = np.tril(np.ones((P, P), np.float32), -1)    # k > q  (far block)
    masks = np.concatenate([maskd, maskf], axis=1).astype(bf16)

    in_maps = []
    for core in range(N_CORES):
        b, g = core // HG, core % HG
        sl = slice(g * GD, (g + 1) * GD)
        cos_b = cos_t[pos[b]]                            # [S, 64]
        sin_b = sin_t[pos[b]]
        cosT = np.tile(cos_b.T, (2, 1)).astype(bf16)     # [128, S]
        sin_sgn = sin_b.T.copy()                         # [64, S]
        sin_sgn[0:32] *= -1.0
        sinTs = np.tile(sin_sgn, (2, 1)).astype(bf16)
        in_maps.append({
            "xT": np.ascontiguousarray(hidden_states[b].T).astype(bf16),
            "wqT": np.ascontiguousarray(wq[sl].T).astype(bf16),
            "wkT": np.ascontiguousarray(wk[sl].T).astype(bf16),
            "wvT": np.ascontiguousarray(wv[sl].T).astype(bf16),
            "woT": np.ascontiguousarray(wo[:, sl].T).astype(bf16),
            "cosT": cosT,
            "sinTs": sinTs,
            "bq2": np.ascontiguousarray(
                bq[sl].reshape(2, P).T).astype(np.float32),
            "bk2": np.ascontiguousarray(
                bk[sl].reshape(2, P).T).astype(np.float32),
            "bq2s": np.ascontiguousarray(
                bq[sl].reshape(2, 2, 2, 32)[:, :, ::-1].reshape(
                    2, P).T).astype(np.float32),
            "bk2s": np.ascontiguousarray(
                bk[sl].reshape(2, 2, 2, 32)[:, :, ::-1].reshape(
                    2, P).T).astype(np.float32),
            "masks": masks,
        })

    res = run_bass_kernel_spmd(nc, in_maps, core_ids=list(range(N_CORES)))

    const_off = (wo @ bv + bo).astype(np.float32)        # host-folded biases
    out = np.empty((B, S, H), dtype=np.float32)
    for b in range(B):
        acc = res.results[b * HG]["out"].astype(np.float32).copy()
        for g in range(1, HG):
            acc += res.results[b * HG + g]["out"]
        out[b] = acc + const_off[None, :]
    return out
